# revision 6
# baseline (speedup 1.0000x reference)
"""DirectionalGAT Trainium2 kernel (8 NeuronCores, SPMD) — mask-bucketed.

Problem (hardcoded shapes): B=4, V=20000, D=10, F=32, OUT=32, mask_index=V.

    summed   = inputs.sum(axis=2)                      # [B,V,F]
    gathered = where(adj==V, 0, summed[b, adj])        # [B,V,D,F]
    X        = (1-mask) * (gathered + initial_states)  # [B,V,D,F]
    t        = (1-mask) * relu(X @ W + b)              # [B,V,D,OUT]
    a        = t @ a_kernel                            # [B,V,D,1]
    coefs    = softmax(a - 1e7*mask, axis=D)
    out      = coefs * t

Sharding: core c -> batch b=c//2, node half h=c%2 (VSH=10000 nodes/core).

Key optimization vs the dense version: slots with mask==1 provably produce
zero output (t==0 there and coefs*0==0), and the gather (SWDGE descriptor
generation on the Pool engine, ~8ns/slot, serial) is the kernel bottleneck.
So the host packs, per node v, only its k_v live directions.  Nodes are
sorted by k_v descending and grouped into 128-node tiles; 4 tiles form a
super-tile with uniform slot width K (= max k in it, host-padded with dummy
slots: ini=0, gather idx=pad row -> X row 0 -> exp(0)=1, corrected by
subtracting the host-provided dummy count from the softmax denominator).
This cuts gathered slots from V*D/2=100k to ~54k per core.

Phase 1 (per-node sum over D) batches 4 node-tiles per DMA (the dense
version was DMA-latency-bound at ~3.7us per 128-node tile).

On-chip layout per super-tile (width K): row-major tiles [128 v, 4*K*F];
a DVE 32x32 block-transpose puts (slot,f) on partitions in 32-blocks so a
block-diagonal weight matmul (lhsT=w4q) computes all four 32-row v-quarters
at once.  Attention dot / softmax-broadcast are small matmuls against
static selector matrices (a4q / o4rep) at base-partition 32*s so the four
128-v subtiles batch into one [128,*] PSUM region.  Output is written
block-transposed and unscattered on the host (host also zero-fills the
masked slots, which are identically zero).

The wire format of the program depends on the mask data (per-super-tile
widths K_SUP), so the Bass program is built per call.
"""

import numpy as np
from contextlib import ExitStack

import concourse.bass as bass
import concourse.bacc as bacc
import concourse.mybir as mybir
import concourse.tile as tile

F32 = mybir.dt.float32
I16 = mybir.dt.int16
EP = 64  # gather table row padding (dma_gather elem must be a 256B multiple)

B, V, D, F, OUT = 4, 20000, 10, 32, 32
P = 128
NCORES = 8
VSH = V // 2          # 10000 nodes per core
GSUB = 4              # 128-v tiles per super-tile
NTILE = 80            # node tiles per core (ceil(10000/128)=79, padded to 80)
NSUP = NTILE // GSUB  # 20 super-tiles
P1B = 4               # phase-1 node tiles batched per DMA


def build_nc(k_sup, num_devices=NCORES, replica_groups=None):
    """Build the Bass program (SPMD-identical across cores).

    k_sup: list of NSUP ints (1..10) — slot width of each super-tile.
    """
    if replica_groups is None:
        replica_groups = [[2 * b, 2 * b + 1] for b in range(num_devices // 2)]
    assert len(k_sup) == NSUP and all(1 <= k <= D for k in k_sup)
    vsh, vfull = VSH, V

    nc = bacc.Bacc("TRN2", num_devices=num_devices)

    ini_total = 512 * 32 * sum(k_sup)           # f32 elements
    idx_total = 128 * 32 * sum(k_sup)           # i16 elements
    x_d = nc.declare_dram_parameter("x", [vsh, D, F], F32, isOutput=False)
    inip_d = nc.declare_dram_parameter("inip", [ini_total], F32, isOutput=False)
    idxw_d = nc.declare_dram_parameter("idxw", [idx_total], I16, isOutput=False)
    ndum_d = nc.declare_dram_parameter("ndum", [NSUP * P, F], F32, isOutput=False)
    w4q_d = nc.declare_dram_parameter("w4q", [P, P], F32, isOutput=False)
    a4q_d = nc.declare_dram_parameter("a4q", [P, 4], F32, isOutput=False)
    o4rep_d = nc.declare_dram_parameter("o4rep", [P, P], F32, isOutput=False)
    bblk_d = nc.declare_dram_parameter("bblk", [P, 1], F32, isOutput=False)
    out_d = nc.declare_dram_parameter("out", [NTILE * P, D * F], F32, isOutput=True)

    with ExitStack() as ctx:
        tc = ctx.enter_context(tile.TileContext(nc))

        dram = ctx.enter_context(tc.tile_pool(name="dram", bufs=1, space="DRAM"))
        cc_in = dram.tile([vsh, EP], F32)
        # +32 zero rows: pad/dummy ids gather row `vfull` -> zeros
        summed_full = dram.tile([vfull + 32, EP], F32)

        cpool = ctx.enter_context(tc.tile_pool(name="const", bufs=1))
        w4q = cpool.tile([P, P], F32, tag="w4q")
        a4q = cpool.tile([P, 4], F32, tag="a4q")
        o4rep = cpool.tile([P, P], F32, tag="o4rep")
        bblk = cpool.tile([P, 1], F32, tag="bblk")
        nc.sync.dma_start(out=w4q[:], in_=w4q_d[:])
        nc.sync.dma_start(out=a4q[:], in_=a4q_d[:])
        nc.sync.dma_start(out=o4rep[:], in_=o4rep_d[:])
        nc.sync.dma_start(out=bblk[:], in_=bblk_d[:])

        # ---- phase 1: per-node sum over D (4 tiles per DMA) ----
        p1pool = ctx.enter_context(tc.tile_pool(name="p1", bufs=3))
        nbat = vsh // (P1B * P)  # 19 full batches -> rows 0..9727
        for bi in range(nbat):
            v0 = bi * P1B * P
            xt = p1pool.tile([P, P1B * D * F], F32, tag="xt")
            sm = p1pool.tile([P, P1B * F], F32, tag="sm")
            nc.sync.dma_start(
                out=xt[:].rearrange("p (c y) -> p c y", c=P1B),
                in_=x_d[v0 : v0 + P1B * P].rearrange(
                    "(c p) d f -> p c (d f)", p=P
                ),
            )
            nc.vector.tensor_reduce(
                out=sm[:].rearrange("p (c f) -> p c f", c=P1B),
                in_=xt[:].rearrange("p (c d f) -> p c f d", d=D, f=F),
                axis=mybir.AxisListType.X,
                op=mybir.AluOpType.add,
            )
            nc.sync.dma_start(
                out=cc_in[v0 : v0 + P1B * P, 0:F].rearrange(
                    "(c p) f -> p c f", p=P
                ),
                in_=sm[:].rearrange("p (c f) -> p c f", c=P1B),
            )
            nc.sync.dma_start(
                out=cc_in[v0 : v0 + P1B * P, F:EP].rearrange(
                    "(c p) f -> p c f", p=P
                ),
                in_=sm[:].rearrange("p (c f) -> p c f", c=P1B),
            )
        # tail: rows 9728..9999 via single tiles (last tile writes new rows only)
        tail0 = nbat * P1B * P
        tails = list(range(tail0, vsh - P + 1, P))
        if tails[-1] != vsh - P:
            tails.append(vsh - P)
        covered = tail0
        for v0 in tails:
            xt = p1pool.tile([P, D * F], F32, tag="xt1")
            sm = p1pool.tile([P, F], F32, tag="sm1")
            nc.sync.dma_start(
                out=xt[:], in_=x_d[v0 : v0 + P].rearrange("v d f -> v (d f)")
            )
            nc.vector.tensor_reduce(
                out=sm[:],
                in_=xt[:].rearrange("p (d f) -> p f d", d=D),
                axis=mybir.AxisListType.X,
                op=mybir.AluOpType.add,
            )
            lo = max(v0, covered)
            nc.sync.dma_start(out=cc_in[lo : v0 + P, 0:F], in_=sm[lo - v0 :, :])
            nc.sync.dma_start(out=cc_in[lo : v0 + P, F:EP], in_=sm[lo - v0 :, :])
            covered = v0 + P

        # zero the pad rows (gathers of pad/dummy ids land here)
        zt = p1pool.tile([32, EP], F32, tag="zt")
        nc.vector.memset(zt[:], 0.0)
        nc.sync.dma_start(out=summed_full[vfull : vfull + 32, :], in_=zt[:])

        # ---- all-gather the summed table within core pairs ----
        nc.gpsimd.collective_compute(
            "AllGather",
            mybir.AluOpType.bypass,
            replica_groups=replica_groups,
            ins=[cc_in[:]],
            outs=[summed_full[0:vfull, :]],
        )

        # ---- phase 2 ----
        kregs = {k: nc.gpsimd.to_reg(512 * k) for k in sorted(set(k_sup))}
        sb = ctx.enter_context(tc.tile_pool(name="sb", bufs=2))
        ps_y = ctx.enter_context(tc.tile_pool(name="psy", bufs=2, space="PSUM"))
        ps_a = ctx.enter_context(tc.tile_pool(name="psa", bufs=2, space="PSUM"))
        ps_s = ctx.enter_context(tc.tile_pool(name="pss", bufs=2, space="PSUM"))

        ini_off = 0
        idx_off = 0
        for sp in range(NSUP):
            k = k_sup[sp]
            kf = k * F            # slot columns per sub
            nidx = 512 * k
            ncol = nidx // 16

            X = sb.tile([P, GSUB * kf], F32, tag="X")
            G = sb.tile([P, GSUB * k * EP], F32, tag="G")
            ixw = sb.tile([P, ncol], I16, tag="ixw")
            Xt = sb.tile([P, GSUB * kf], F32, tag="Xt")
            tb = sb.tile([P, GSUB * kf], F32, tag="tb")
            ob = sb.tile([P, GSUB * kf], F32, tag="ob")
            nd = sb.tile([P, F], F32, tag="nd")
            E4 = sb.tile([P, kf], F32, tag="E4")
            szm = sb.tile([P, kf], F32, tag="szm")
            sum4 = sb.tile([P, F], F32, tag="sum4")
            r4 = sb.tile([P, F], F32, tag="r4")

            AT4 = ps_a.tile([P, kf], F32, tag="AT4")
            nc.vector.memset(AT4[:], 0.0)

            nc.sync.dma_start(
                out=ixw[:],
                in_=idxw_d[idx_off : idx_off + P * ncol].rearrange(
                    "(p c) -> p c", p=P
                ),
            )
            nc.sync.dma_start(
                out=X[:].rearrange("p (s c) -> p s c", s=GSUB),
                in_=inip_d[ini_off : ini_off + 512 * kf].rearrange(
                    "(s p c) -> p s c", s=GSUB, p=P
                ),
            )
            nc.sync.dma_start(out=nd[:], in_=ndum_d[sp * P : (sp + 1) * P, :])
            idx_off += P * ncol
            ini_off += 512 * kf

            # one batched gather for the whole super-tile
            nc.gpsimd.dma_gather(
                out_ap=G[:].rearrange("p (c e) -> p c e", e=EP),
                in_ap=summed_full[:],
                idxs_ap=ixw[:],
                num_idxs=nidx,
                num_idxs_reg=kregs[k],
                elem_size=EP,
                single_packet=False,
            )
            # X += gathered (first F of each padded row)
            nc.vector.tensor_tensor(
                out=X[:].rearrange("p (c f) -> p c f", f=F),
                in0=X[:].rearrange("p (c f) -> p c f", f=F),
                in1=G[:].rearrange("p (c e) -> p c e", e=EP)[:, :, 0:F],
                op=mybir.AluOpType.add,
            )
            # block-transpose: partitions (quarter,f), free (slot, v')
            nc.vector.transpose(out=Xt[:], in_=X[:])

            for s in range(GSUB):
                y = ps_y.tile([P, kf], F32, tag="y")
                nc.tensor.matmul(
                    out=y[:],
                    lhsT=w4q[:],
                    rhs=Xt[:, s * kf : (s + 1) * kf],
                    start=True,
                    stop=True,
                )
                nc.scalar.activation(
                    out=tb[:, s * kf : (s + 1) * kf],
                    in_=y[:],
                    func=mybir.ActivationFunctionType.Relu,
                    bias=bblk[:],
                )
                nc.tensor.matmul(
                    out=AT4[32 * s : 32 * s + 4, :],
                    lhsT=a4q[:],
                    rhs=tb[:, s * kf : (s + 1) * kf],
                    start=True,
                    stop=True,
                    tile_position=(0, 32 * s),
                )

            # softmax over slots, batched over the 4 subtiles
            nc.scalar.activation(
                out=E4[:], in_=AT4[:], func=mybir.ActivationFunctionType.Exp
            )
            nc.vector.tensor_reduce(
                out=sum4[:],
                in_=E4[:].rearrange("p (d j) -> p j d", d=k),
                axis=mybir.AxisListType.X,
                op=mybir.AluOpType.add,
            )
            # subtract dummy-slot contributions (exp(0)=1 each)
            nc.vector.tensor_tensor(
                out=sum4[:], in0=sum4[:], in1=nd[:], op=mybir.AluOpType.subtract
            )
            nc.vector.tensor_scalar_max(out=sum4[:], in0=sum4[:], scalar1=1e-30)
            nc.vector.reciprocal(out=r4[:], in_=sum4[:])
            nc.vector.tensor_tensor(
                out=szm[:].rearrange("p (d j) -> p j d", d=k),
                in0=E4[:].rearrange("p (d j) -> p j d", d=k),
                in1=r4[:].to_broadcast([P, F, k]),
                op=mybir.AluOpType.mult,
            )

            for s in range(GSUB):
                S = ps_s.tile([P, kf], F32, tag="S")
                nc.tensor.matmul(
                    out=S[:],
                    lhsT=o4rep[32 * s : 32 * s + 4, :],
                    rhs=szm[32 * s : 32 * s + 4, :],
                    start=True,
                    stop=True,
                    tile_position=(32 * s, 0),
                )
                nc.vector.tensor_tensor(
                    out=ob[:, s * kf : (s + 1) * kf],
                    in0=tb[:, s * kf : (s + 1) * kf],
                    in1=S[:],
                    op=mybir.AluOpType.mult,
                )
                gi = sp * GSUB + s
                nc.sync.dma_start(
                    out=out_d[gi * P : (gi + 1) * P, 0:kf],
                    in_=ob[:, s * kf : (s + 1) * kf],
                )

    nc.finalize()
    return nc


# ---------------- host side ----------------


def _consts(W_kernel, W_bias, a_kernel):
    w4q = np.zeros((P, P), np.float32)
    a4q = np.zeros((P, 4), np.float32)
    o4rep = np.zeros((P, P), np.float32)
    bblk = np.zeros((P, 1), np.float32)
    for g in range(4):
        w4q[32 * g : 32 * g + 32, 32 * g : 32 * g + 32] = W_kernel
        a4q[32 * g : 32 * g + 32, g] = a_kernel[:, 0]
        bblk[32 * g : 32 * g + 32, 0] = W_bias
        for s in range(4):
            o4rep[32 * s + g, 32 * g : 32 * g + 32] = 1.0
    return w4q, a4q, o4rep, bblk


def plan_core(mask_c):
    """Sort nodes by live-direction count (desc); return per-core plan."""
    k_v = (mask_c == 0).sum(1).astype(np.int64)            # [VSH]
    order = np.argsort(-k_v, kind="stable")
    live = order[k_v[order] > 0]
    if len(live) == 0:
        live = order[:1]
    pad = NTILE * P - len(live)
    livep = np.concatenate([live, np.repeat(live[-1], pad)])
    vt = livep.reshape(NTILE, P)                            # [tile, p]
    # per node: live d's ascending, padded with D (sentinel)
    dmat = np.where(mask_c == 0, np.arange(D)[None, :], D)
    dmat = np.sort(dmat, axis=1)                            # [VSH, D]
    k_sup = [int(k_v[vt[GSUB * j, 0]]) for j in range(NSUP)]
    k_sup = [max(k, 1) for k in k_sup]
    return {"k_v": k_v, "vt": vt, "dmat": dmat, "k_sup": k_sup}


def pack_core(ini_c, adj_c, plan, k_sup):
    """Build device input arrays for one core under global widths k_sup."""
    k_v, vt, dmat = plan["k_v"], plan["vt"], plan["dmat"]
    ini_parts, idx_parts, ndum = [], [], np.zeros((NSUP * P, F), np.float32)
    for j in range(NSUP):
        k = k_sup[j]
        vtile = vt[GSUB * j : GSUB * (j + 1)]               # [4, 128]
        ds = dmat[vtile][:, :, :k]                          # [4, 128, k]
        dummy = ds >= D
        dcl = np.minimum(ds, D - 1)
        adj_slot = np.where(dummy, V, adj_c[vtile[:, :, None], dcl])
        ini_slot = ini_c[vtile[:, :, None], dcl, :] * (~dummy[:, :, :, None])
        ini_parts.append(ini_slot.reshape(-1).astype(np.float32))
        flat = adj_slot.transpose(0, 2, 1).reshape(-1)      # (s, jj, p)
        wrapped = flat.reshape(-1, 16).T                    # [16, ncol]
        idx_parts.append(
            np.tile(wrapped, (8, 1)).reshape(-1).astype(np.int16)
        )
        nd = (k - k_v[vtile]).astype(np.float32)            # [4, 128]
        ndj = np.zeros((P, F), np.float32)
        for s in range(GSUB):
            q = np.arange(P) // 32                          # quarter of p
            ndj[32 * s + q, np.arange(P) % 32] = nd[s]
        ndum[j * P : (j + 1) * P] = ndj
    return {
        "inip": np.concatenate(ini_parts),
        "idxw": np.concatenate(idx_parts),
        "ndum": ndum,
    }


def unpack_core(out_dev, plan, k_sup, out_half):
    """Scatter device output back to [VSH, D, F] (out_half pre-zeroed)."""
    k_v, vt, dmat = plan["k_v"], plan["vt"], plan["dmat"]
    for t in range(NTILE):
        k = k_sup[t // GSUB]
        blk = out_dev[t * P : (t + 1) * P, : k * F]
        unb = (
            blk.reshape(4, 32, k, 32).transpose(0, 3, 2, 1).reshape(P, k, F)
        )
        vtile = vt[t]                                       # [128]
        kk = np.minimum(k_v[vtile], k)
        valid = np.arange(k)[None, :] < kk[:, None]         # [128, k]
        ds = dmat[vtile][:, :k]
        vsel = np.broadcast_to(vtile[:, None], (P, k))
        out_half[vsel[valid], ds[valid], :] = unb[valid, :]


def kernel(
    inputs,
    initial_states,
    mask,
    W_kernel,
    W_bias,
    a_kernel,
    adj_lst,
    mask_index,
):
    from concourse.bass_utils import run_bass_kernel_spmd

    inputs = np.asarray(inputs, np.float32)
    initial_states = np.asarray(initial_states, np.float32)
    mask = np.asarray(mask, np.float32)
    adj = np.asarray(adj_lst)
    # pad ids (== mask_index) gather the zeroed pad row at V
    adj = np.where(adj == np.asarray(mask_index), V, adj).astype(np.int32)
    w4q, a4q, o4rep, bblk = _consts(
        np.asarray(W_kernel, np.float32),
        np.asarray(W_bias, np.float32),
        np.asarray(a_kernel, np.float32),
    )

    plans = []
    for c in range(NCORES):
        b, h = c // 2, c % 2
        sl = slice(h * VSH, (h + 1) * VSH)
        plans.append(plan_core(mask[b, sl]))
    k_sup = [
        max(plans[c]["k_sup"][j] for c in range(NCORES)) for j in range(NSUP)
    ]

    nc = build_nc(k_sup)

    in_maps = []
    for c in range(NCORES):
        b, h = c // 2, c % 2
        sl = slice(h * VSH, (h + 1) * VSH)
        pk = pack_core(initial_states[b, sl], adj[b, sl], plans[c], k_sup)
        in_maps.append(
            {
                "x": np.ascontiguousarray(inputs[b, sl]),
                "inip": pk["inip"],
                "idxw": pk["idxw"],
                "ndum": pk["ndum"],
                "w4q": w4q,
                "a4q": a4q,
                "o4rep": o4rep,
                "bblk": bblk,
            }
        )

    res = run_bass_kernel_spmd(nc, in_maps, list(range(NCORES)))
    out = np.zeros((B, V, D, OUT), np.float32)
    for c in range(NCORES):
        b, h = c // 2, c % 2
        unpack_core(
            res.results[c]["out"], plans[c], k_sup,
            out[b, h * VSH : (h + 1) * VSH],
        )
    return out


# revision 15
# speedup vs baseline: 1.0993x; 1.0993x over previous
"""DirectionalGAT Trainium2 kernel (8 NeuronCores, SPMD) — mask-bucketed.

Problem (hardcoded shapes): B=4, V=20000, D=10, F=32, OUT=32, mask_index=V.

    summed   = inputs.sum(axis=2)                      # [B,V,F]
    gathered = where(adj==V, 0, summed[b, adj])        # [B,V,D,F]
    X        = (1-mask) * (gathered + initial_states)  # [B,V,D,F]
    t        = (1-mask) * relu(X @ W + b)              # [B,V,D,OUT]
    a        = t @ a_kernel                            # [B,V,D,1]
    coefs    = softmax(a - 1e7*mask, axis=D)
    out      = coefs * t

Sharding: core c -> batch b=c//2, node half h=c%2 (VSH=10000 nodes/core).

Key optimization vs the dense version: slots with mask==1 provably produce
zero output (t==0 there and coefs*0==0), and the gather (SWDGE descriptor
generation on the Pool engine, ~8ns/slot, serial) is the kernel bottleneck.
So the host packs, per node v, only its k_v live directions.  Nodes are
sorted by k_v descending and grouped into 128-node tiles; 4 tiles form a
super-tile with uniform slot width K (= max k in it, host-padded with dummy
slots: ini=0, gather idx=pad row -> X row 0 -> exp(0)=1, corrected by
subtracting the host-provided dummy count from the softmax denominator).
This cuts gathered slots from V*D/2=100k to ~54k per core.

Phase 1 (per-node sum over D) batches 4 node-tiles per DMA (the dense
version was DMA-latency-bound at ~3.7us per 128-node tile).

On-chip layout per super-tile (width K): row-major tiles [128 v, 4*K*F];
a DVE 32x32 block-transpose puts (slot,f) on partitions in 32-blocks so a
block-diagonal weight matmul (lhsT=w4q) computes all four 32-row v-quarters
at once.  Attention dot / softmax-broadcast are small matmuls against
static selector matrices (a4q / o4rep) at base-partition 32*s so the four
128-v subtiles batch into one [128,*] PSUM region.  Output is written
block-transposed and unscattered on the host (host also zero-fills the
masked slots, which are identically zero).

The wire format of the program depends on the mask data (per-super-tile
widths K_SUP), so the Bass program is built per call.
"""

import numpy as np
from contextlib import ExitStack

import concourse.bass as bass
import concourse.bacc as bacc
import concourse.mybir as mybir
import concourse.tile as tile

F32 = mybir.dt.float32
I16 = mybir.dt.int16
EP = 64  # gather table row padding (dma_gather elem must be a 256B multiple)

B, V, D, F, OUT = 4, 20000, 10, 32, 32
P = 128
NCORES = 8
VSH = V // 2          # 10000 nodes per core
GSUB = 4              # 128-v tiles per super-tile
NTILE = 80            # node tiles per core (ceil(10000/128)=79, padded to 80)
NSUP = NTILE // GSUB  # 20 super-tiles
P1B = 4               # phase-1 node tiles batched per DMA


def build_nc(k_sup, num_devices=NCORES, replica_groups=None):
    """Build the Bass program (SPMD-identical across cores).

    k_sup: list of NSUP ints (1..10) — slot width of each super-tile.
    """
    if replica_groups is None:
        replica_groups = [[2 * b, 2 * b + 1] for b in range(num_devices // 2)]
    assert len(k_sup) == NSUP and all(1 <= k <= D for k in k_sup)
    vsh, vfull = VSH, V

    nc = bacc.Bacc("TRN2", num_devices=num_devices, num_swdge_queues=4)

    ini_total = 512 * 32 * sum(k_sup)           # f32 elements
    idx_total = 128 * 32 * sum(k_sup)           # i16 elements
    x_d = nc.declare_dram_parameter("x", [vsh, D, F], F32, isOutput=False)
    inip_d = nc.declare_dram_parameter("inip", [ini_total], F32, isOutput=False)
    idxw_d = nc.declare_dram_parameter("idxw", [idx_total], I16, isOutput=False)
    ndum_d = nc.declare_dram_parameter("ndum", [NSUP * P, F], F32, isOutput=False)
    w4q_d = nc.declare_dram_parameter("w4q", [P, P], F32, isOutput=False)
    a4q_d = nc.declare_dram_parameter("a4q", [P, 4], F32, isOutput=False)
    o4rep_d = nc.declare_dram_parameter("o4rep", [P, P], F32, isOutput=False)
    bblk_d = nc.declare_dram_parameter("bblk", [P, 1], F32, isOutput=False)
    out_d = nc.declare_dram_parameter("out", [NTILE * P, D * F], F32, isOutput=True)

    with ExitStack() as ctx:
        tc = ctx.enter_context(tile.TileContext(nc))

        dram = ctx.enter_context(tc.tile_pool(name="dram", bufs=1, space="DRAM"))
        cc_in = dram.tile([vsh, F], F32)
        summed_c = dram.tile([vfull, F], F32)
        # +32 zero rows: pad/dummy ids gather row `vfull` -> zeros.
        # Rows are 256B (dma_gather elem) but only cols 0:F are ever written;
        # the gathered upper half is never read.
        summed_full = dram.tile([vfull + 32, EP], F32)

        cpool = ctx.enter_context(tc.tile_pool(name="const", bufs=1))
        w4q = cpool.tile([P, P], F32, tag="w4q")
        a4q = cpool.tile([P, 4], F32, tag="a4q")
        o4rep = cpool.tile([P, P], F32, tag="o4rep")
        bblk = cpool.tile([P, 1], F32, tag="bblk")
        nc.sync.dma_start(out=w4q[:], in_=w4q_d[:])
        nc.sync.dma_start(out=a4q[:], in_=a4q_d[:])
        nc.sync.dma_start(out=o4rep[:], in_=o4rep_d[:])
        nc.sync.dma_start(out=bblk[:], in_=bblk_d[:])

        # ---- phase 1: per-node sum over D (4 tiles per DMA) ----
        p1pool = ctx.enter_context(tc.tile_pool(name="p1", bufs=3))
        nbat = vsh // (P1B * P)  # 19 full batches -> rows 0..9727
        for bi in range(nbat):
            v0 = bi * P1B * P
            xt = p1pool.tile([P, P1B * D * F], F32, tag="xt")
            sm = p1pool.tile([P, P1B * F], F32, tag="sm")
            nc.sync.dma_start(
                out=xt[:].rearrange("p (c y) -> p c y", c=P1B),
                in_=x_d[v0 : v0 + P1B * P].rearrange(
                    "(c p) d f -> p c (d f)", p=P
                ),
            )
            nc.vector.tensor_reduce(
                out=sm[:].rearrange("p (c f) -> p c f", c=P1B),
                in_=xt[:].rearrange("p (c d f) -> p c f d", d=D, f=F),
                axis=mybir.AxisListType.X,
                op=mybir.AluOpType.add,
            )
            nc.sync.dma_start(
                out=cc_in[v0 : v0 + P1B * P, :].rearrange(
                    "(c p) f -> p c f", p=P
                ),
                in_=sm[:].rearrange("p (c f) -> p c f", c=P1B),
            )
        # tail: rows 9728..9999 via single tiles (last tile writes new rows only)
        tail0 = nbat * P1B * P
        tails = list(range(tail0, vsh - P + 1, P))
        if tails[-1] != vsh - P:
            tails.append(vsh - P)
        covered = tail0
        for v0 in tails:
            xt = p1pool.tile([P, D * F], F32, tag="xt1")
            sm = p1pool.tile([P, F], F32, tag="sm1")
            nc.sync.dma_start(
                out=xt[:], in_=x_d[v0 : v0 + P].rearrange("v d f -> v (d f)")
            )
            nc.vector.tensor_reduce(
                out=sm[:],
                in_=xt[:].rearrange("p (d f) -> p f d", d=D),
                axis=mybir.AxisListType.X,
                op=mybir.AluOpType.add,
            )
            lo = max(v0, covered)
            nc.sync.dma_start(out=cc_in[lo : v0 + P, :], in_=sm[lo - v0 :, :])
            covered = v0 + P

        # zero the pad rows (gathers of pad/dummy ids land here)
        zt = p1pool.tile([32, EP], F32, tag="zt")
        nc.vector.memset(zt[:], 0.0)
        nc.sync.dma_start(out=summed_full[vfull : vfull + 32, :], in_=zt[:])

        # ---- all-gather the summed table within core pairs ----
        # Exchange the compact [V,32] table (collective outputs must be
        # contiguous), then expand locally into the 256B-strided gather rows.
        nc.gpsimd.collective_compute(
            "AllGather",
            mybir.AluOpType.bypass,
            replica_groups=replica_groups,
            ins=[cc_in[:]],
            outs=[summed_c[:]],
        )
        nc.sync.dma_start(out=summed_full[0:vfull, 0:F], in_=summed_c[:])

        # ---- phase 2 ----
        kregs = {k: nc.gpsimd.to_reg(512 * k) for k in sorted(set(k_sup))}
        sb = ctx.enter_context(tc.tile_pool(name="sb", bufs=2))
        ps_y = ctx.enter_context(tc.tile_pool(name="psy", bufs=2, space="PSUM"))
        ps_a = ctx.enter_context(tc.tile_pool(name="psa", bufs=2, space="PSUM"))
        ps_s = ctx.enter_context(tc.tile_pool(name="pss", bufs=2, space="PSUM"))

        ini_off = 0
        idx_off = 0
        for sp in range(NSUP):
            k = k_sup[sp]
            kf = k * F            # slot columns per sub
            nidx = 512 * k
            ncol = nidx // 16

            X = sb.tile([P, GSUB * kf], F32, tag="X")
            G = sb.tile([P, GSUB * k * EP], F32, tag="G")
            ixw = sb.tile([P, ncol], I16, tag="ixw")
            Xt = sb.tile([P, GSUB * kf], F32, tag="Xt")
            tb = sb.tile([P, GSUB * kf], F32, tag="tb")
            ob = sb.tile([P, GSUB * kf], F32, tag="ob")
            nd = sb.tile([P, F], F32, tag="nd")
            E4 = sb.tile([P, kf], F32, tag="E4")
            szm = sb.tile([P, kf], F32, tag="szm")
            sum4 = sb.tile([P, F], F32, tag="sum4")
            r4 = sb.tile([P, F], F32, tag="r4")

            AT4 = ps_a.tile([P, kf], F32, tag="AT4")
            nc.vector.memset(AT4[:], 0.0)

            nc.sync.dma_start(
                out=ixw[:],
                in_=idxw_d[idx_off : idx_off + P * ncol].rearrange(
                    "(p c) -> p c", p=P
                ),
            )
            nc.sync.dma_start(
                out=X[:].rearrange("p (s c) -> p s c", s=GSUB),
                in_=inip_d[ini_off : ini_off + 512 * kf].rearrange(
                    "(s p c) -> p s c", s=GSUB, p=P
                ),
            )
            nc.sync.dma_start(out=nd[:], in_=ndum_d[sp * P : (sp + 1) * P, :])
            idx_off += P * ncol
            ini_off += 512 * kf

            # one batched gather for the whole super-tile; round-robin the
            # SWDGE queues so ring drains overlap the next generation
            nc.gpsimd.dma_gather(
                out_ap=G[:].rearrange("p (c e) -> p c e", e=EP),
                in_ap=summed_full[:],
                idxs_ap=ixw[:],
                num_idxs=nidx,
                num_idxs_reg=kregs[k],
                elem_size=EP,
                single_packet=False,
                queue_num=sp % 4,
            )
            # X += gathered (first F of each padded row)
            nc.vector.tensor_tensor(
                out=X[:].rearrange("p (c f) -> p c f", f=F),
                in0=X[:].rearrange("p (c f) -> p c f", f=F),
                in1=G[:].rearrange("p (c e) -> p c e", e=EP)[:, :, 0:F],
                op=mybir.AluOpType.add,
            )
            # block-transpose: partitions (quarter,f), free (slot, v')
            nc.vector.transpose(out=Xt[:], in_=X[:])

            for s in range(GSUB):
                y = ps_y.tile([P, kf], F32, tag="y")
                nc.tensor.matmul(
                    out=y[:],
                    lhsT=w4q[:],
                    rhs=Xt[:, s * kf : (s + 1) * kf],
                    start=True,
                    stop=True,
                )
                nc.scalar.activation(
                    out=tb[:, s * kf : (s + 1) * kf],
                    in_=y[:],
                    func=mybir.ActivationFunctionType.Relu,
                    bias=bblk[:],
                )
                nc.tensor.matmul(
                    out=AT4[32 * s : 32 * s + 4, :],
                    lhsT=a4q[:],
                    rhs=tb[:, s * kf : (s + 1) * kf],
                    start=True,
                    stop=True,
                    tile_position=(0, 32 * s),
                )

            # softmax over slots, batched over the 4 subtiles
            nc.scalar.activation(
                out=E4[:], in_=AT4[:], func=mybir.ActivationFunctionType.Exp
            )
            nc.vector.tensor_reduce(
                out=sum4[:],
                in_=E4[:].rearrange("p (d j) -> p j d", d=k),
                axis=mybir.AxisListType.X,
                op=mybir.AluOpType.add,
            )
            # subtract dummy-slot contributions (exp(0)=1 each)
            nc.vector.tensor_tensor(
                out=sum4[:], in0=sum4[:], in1=nd[:], op=mybir.AluOpType.subtract
            )
            nc.vector.tensor_scalar_max(out=sum4[:], in0=sum4[:], scalar1=1e-30)
            nc.vector.reciprocal(out=r4[:], in_=sum4[:])
            nc.vector.tensor_tensor(
                out=szm[:].rearrange("p (d j) -> p j d", d=k),
                in0=E4[:].rearrange("p (d j) -> p j d", d=k),
                in1=r4[:].to_broadcast([P, F, k]),
                op=mybir.AluOpType.mult,
            )

            for s in range(GSUB):
                S = ps_s.tile([P, kf], F32, tag="S")
                nc.tensor.matmul(
                    out=S[:],
                    lhsT=o4rep[32 * s : 32 * s + 4, :],
                    rhs=szm[32 * s : 32 * s + 4, :],
                    start=True,
                    stop=True,
                    tile_position=(32 * s, 0),
                )
                nc.vector.tensor_tensor(
                    out=ob[:, s * kf : (s + 1) * kf],
                    in0=tb[:, s * kf : (s + 1) * kf],
                    in1=S[:],
                    op=mybir.AluOpType.mult,
                )
                gi = sp * GSUB + s
                nc.sync.dma_start(
                    out=out_d[gi * P : (gi + 1) * P, 0:kf],
                    in_=ob[:, s * kf : (s + 1) * kf],
                )

    nc.finalize()
    return nc


# ---------------- host side ----------------


def _consts(W_kernel, W_bias, a_kernel):
    w4q = np.zeros((P, P), np.float32)
    a4q = np.zeros((P, 4), np.float32)
    o4rep = np.zeros((P, P), np.float32)
    bblk = np.zeros((P, 1), np.float32)
    for g in range(4):
        w4q[32 * g : 32 * g + 32, 32 * g : 32 * g + 32] = W_kernel
        a4q[32 * g : 32 * g + 32, g] = a_kernel[:, 0]
        bblk[32 * g : 32 * g + 32, 0] = W_bias
        for s in range(4):
            o4rep[32 * s + g, 32 * g : 32 * g + 32] = 1.0
    return w4q, a4q, o4rep, bblk


def plan_core(mask_c):
    """Sort nodes by live-direction count (desc); return per-core plan."""
    k_v = (mask_c == 0).sum(1).astype(np.int64)            # [VSH]
    order = np.argsort(-k_v, kind="stable")
    live = order[k_v[order] > 0]
    if len(live) == 0:
        live = order[:1]
    pad = NTILE * P - len(live)
    livep = np.concatenate([live, np.repeat(live[-1], pad)])
    vt = livep.reshape(NTILE, P)                            # [tile, p]
    # per node: live d's ascending, padded with D (sentinel)
    dmat = np.where(mask_c == 0, np.arange(D)[None, :], D)
    dmat = np.sort(dmat, axis=1)                            # [VSH, D]
    k_sup = [int(k_v[vt[GSUB * j, 0]]) for j in range(NSUP)]
    k_sup = [max(k, 1) for k in k_sup]
    return {"k_v": k_v, "vt": vt, "dmat": dmat, "k_sup": k_sup}


def pack_core(ini_c, adj_c, plan, k_sup):
    """Build device input arrays for one core under global widths k_sup."""
    k_v, vt, dmat = plan["k_v"], plan["vt"], plan["dmat"]
    ini_parts, idx_parts, ndum = [], [], np.zeros((NSUP * P, F), np.float32)
    for j in range(NSUP):
        k = k_sup[j]
        vtile = vt[GSUB * j : GSUB * (j + 1)]               # [4, 128]
        ds = dmat[vtile][:, :, :k]                          # [4, 128, k]
        dummy = ds >= D
        dcl = np.minimum(ds, D - 1)
        adj_slot = np.where(dummy, V, adj_c[vtile[:, :, None], dcl])
        ini_slot = ini_c[vtile[:, :, None], dcl, :] * (~dummy[:, :, :, None])
        ini_parts.append(ini_slot.reshape(-1).astype(np.float32))
        flat = adj_slot.transpose(0, 2, 1).reshape(-1)      # (s, jj, p)
        wrapped = flat.reshape(-1, 16).T                    # [16, ncol]
        idx_parts.append(
            np.tile(wrapped, (8, 1)).reshape(-1).astype(np.int16)
        )
        nd = (k - k_v[vtile]).astype(np.float32)            # [4, 128]
        ndj = np.zeros((P, F), np.float32)
        for s in range(GSUB):
            q = np.arange(P) // 32                          # quarter of p
            ndj[32 * s + q, np.arange(P) % 32] = nd[s]
        ndum[j * P : (j + 1) * P] = ndj
    return {
        "inip": np.concatenate(ini_parts),
        "idxw": np.concatenate(idx_parts),
        "ndum": ndum,
    }


def unpack_core(out_dev, plan, k_sup, out_half):
    """Scatter device output back to [VSH, D, F] (out_half pre-zeroed)."""
    k_v, vt, dmat = plan["k_v"], plan["vt"], plan["dmat"]
    for t in range(NTILE):
        k = k_sup[t // GSUB]
        blk = out_dev[t * P : (t + 1) * P, : k * F]
        unb = (
            blk.reshape(4, 32, k, 32).transpose(0, 3, 2, 1).reshape(P, k, F)
        )
        vtile = vt[t]                                       # [128]
        kk = np.minimum(k_v[vtile], k)
        valid = np.arange(k)[None, :] < kk[:, None]         # [128, k]
        ds = dmat[vtile][:, :k]
        vsel = np.broadcast_to(vtile[:, None], (P, k))
        out_half[vsel[valid], ds[valid], :] = unb[valid, :]


def kernel(
    inputs,
    initial_states,
    mask,
    W_kernel,
    W_bias,
    a_kernel,
    adj_lst,
    mask_index,
):
    from concourse.bass_utils import run_bass_kernel_spmd

    inputs = np.asarray(inputs, np.float32)
    initial_states = np.asarray(initial_states, np.float32)
    mask = np.asarray(mask, np.float32)
    adj = np.asarray(adj_lst)
    # pad ids (== mask_index) gather the zeroed pad row at V
    adj = np.where(adj == np.asarray(mask_index), V, adj).astype(np.int32)
    w4q, a4q, o4rep, bblk = _consts(
        np.asarray(W_kernel, np.float32),
        np.asarray(W_bias, np.float32),
        np.asarray(a_kernel, np.float32),
    )

    plans = []
    for c in range(NCORES):
        b, h = c // 2, c % 2
        sl = slice(h * VSH, (h + 1) * VSH)
        plans.append(plan_core(mask[b, sl]))
    k_sup = [
        max(plans[c]["k_sup"][j] for c in range(NCORES)) for j in range(NSUP)
    ]

    nc = build_nc(k_sup)

    in_maps = []
    for c in range(NCORES):
        b, h = c // 2, c % 2
        sl = slice(h * VSH, (h + 1) * VSH)
        pk = pack_core(initial_states[b, sl], adj[b, sl], plans[c], k_sup)
        in_maps.append(
            {
                "x": np.ascontiguousarray(inputs[b, sl]),
                "inip": pk["inip"],
                "idxw": pk["idxw"],
                "ndum": pk["ndum"],
                "w4q": w4q,
                "a4q": a4q,
                "o4rep": o4rep,
                "bblk": bblk,
            }
        )

    res = run_bass_kernel_spmd(nc, in_maps, list(range(NCORES)))
    out = np.zeros((B, V, D, OUT), np.float32)
    for c in range(NCORES):
        b, h = c // 2, c % 2
        unpack_core(
            res.results[c]["out"], plans[c], k_sup,
            out[b, h * VSH : (h + 1) * VSH],
        )
    return out
